# revision 45
# baseline (speedup 1.0000x reference)
"""Trainium2 Bass kernel for nn_Attention_44143673868291.

Data-parallel over batch: core b handles x[b] / pos_embed[b] entirely
(B == n_cores == 8, no collectives).

The kernel is softmax-exp bound: the P-pass (exp of 8*2048*2048 scores +
the psum->sbuf move, 33.5M elements/core) is the only irreducible
per-element work, and only ACT and DVE can read PSUM (1 elem/cycle/lane
each). So the pass is SPLIT across both engines: ACT runs exact exp;
DVE runs a Schraudolph exp (int16 round of a*S+b, bit-read as bf16,
~+-3% sawtooth that largely cancels in P/r). Everything else is arranged
to stay off those two engines' critical path.

Per-core dataflow (N=2048 tokens, DIM=256, H=8 heads, HD=32):
  1. LayerNorm stats on x tiles [128t, 256] (bn_stats/bn_aggr on DVE);
     rstd = 1/sqrt(var+eps) via fp32 bit-trick + 2 Newton steps on DVE,
     so ACT only ever runs Copy+Exp -> one table set, zero reloads
  2. PE-transpose centered tiles -> xnT [256d, N] bf16 (gamma/beta are
     folded into Wq/Wk/Wv/biases host-side; psum->sbuf copy on ACT)
  3. qT/kT = Wq^T/Wk^T @ [xnT; posT] -> [256d, N] bf16; v = xn @ Wv
     (all biases added on PE via K=1 ones matmuls; q/k/v psum->sbuf
     copies on ACT, keeping DVE free for LN centering)
  4. Attention in S^T layout: for each head group g (4 heads), i-block:
       S^T[j-tile, i] = kT^T.T @ qT  (K=32, 4 heads row-packed into two
           2-head psum tiles so exp(stA) overlaps matmuls into stB)
       P^T = exp(SCALE*S^T): dve_x of 32 half-tiles per (g,ib) on DVE
           (Schraudolph), rest on ACT (exact)
       O^T[d,i] += v_chunk.T @ P^T_chunk;  r[i] += ones32.T @ P^T_chunk
           (M=64 fused [v|ones] lhsT, 4 heads col/bank-packed; banks are
           zero-filled by PE start=True matmuls with an all-zero lhsT)
  5. R = recip(bank) on DVE; partition-aligned via SBUF->SBUF DMA shift;
     outT = O^T * R -> bf16
  6. out = x + outT^T @ Wp + bp (bf16 weights, K=1 bias matmul; residual
     added on PE via exact fp32 identity matmul, ACT does the final copy)
"""

import os
import sys

_REPO = "/opt/trn_rl_repo"
if _REPO not in sys.path:
    sys.path.insert(0, _REPO)

import numpy as np
import ml_dtypes

import concourse.bass as bass
import concourse.mybir as mybir
import concourse.bacc as bacc
import concourse.tile as tile
from concourse import bass_utils

F32 = mybir.dt.float32
BF16 = mybir.dt.bfloat16
BF16_NP = ml_dtypes.bfloat16
Alu = mybir.AluOpType
Act = mybir.ActivationFunctionType

B, N, DIM, POS, H = 8, 2048, 256, 128, 8
QK = DIM + POS  # 384
HD = DIM // H   # 32
SCALE = HD ** -0.5
EPS = 1e-5
IBS = 512       # attention i-block (columns per psum bank, fp32)
I16 = mybir.dt.int16
# Schraudolph exp on DVE: i16 = rint(EXPA*S + EXPB) bit-read as bf16
# approximates exp(SCALE*S) within ~+-3.3% (bias -6.5 centers the sawtooth;
# the systematic part cancels in P/r since r sums the same approx values).
EXPA = float(SCALE * 128.0 / np.log(2.0))
EXPB = float(127.0 * 128.0 - 6.5)
I32 = mybir.dt.int32
RSQRT_MAGIC = 0x5F3759DF


def build_nc(n=N, use_recip_approx=True, debug_dump=False, _x=(), repeat=1,
             dve_x=14):
    """Emit the single-core program (same program for all 8 cores)."""
    TT = n // 128    # token tiles
    IB = n // IBS    # i-blocks
    JT = n // 128    # j-tiles
    n512 = n // 512
    # exp work split: of the 2*JT st half-tiles per (g, ib), dve_x go to
    # DVE (Schraudolph) and the rest to ACT (exact exp), spread evenly so
    # both engines stay busy through the loop
    nslots = 2 * JT
    dve_mark = [(s + 1) * dve_x // nslots > s * dve_x // nslots
                for s in range(nslots)]
    dve_a = {jt for jt in range(JT) if dve_mark[2 * jt]}
    dve_b = {jt for jt in range(JT) if dve_mark[2 * jt + 1]}

    nc = bacc.Bacc("TRN2", target_bir_lowering=False, debug=False)

    d = lambda name, shape, dt: nc.dram_tensor(name, shape, dt, kind="ExternalInput").ap()
    x_d = d("x", [n, DIM], F32)
    posT_d = d("posT", [POS, n], BF16)
    wq_d = d("wq", [QK, DIM], BF16)
    wk_d = d("wk", [QK, DIM], BF16)
    wv_d = d("wv", [DIM, DIM], BF16)
    wp_d = [d(f"wp{p}", [128, DIM], BF16) for p in range(4)]
    bqT_d = d("bqT", [1, DIM], BF16)
    bkT_d = d("bkT", [1, DIM], BF16)
    bv_d = d("bv", [1, DIM], BF16)
    bp_d = d("bp", [1, DIM], BF16)
    onerb_d = d("ones_row_bf", [1, 128], BF16)
    oner512_d = d("ones_row_512", [1, 512], BF16)
    i128_d = d("i128", [128, 128], F32)
    zeroc_d = d("zeroc", [128, 1], F32)
    out_d = nc.dram_tensor("out", [n, DIM], F32, kind="ExternalOutput").ap()
    dbg = {}
    if debug_dump:
        for nm, dt_ in [("dbg_qT0", BF16), ("dbg_qT1", BF16), ("dbg_kT0", BF16),
                        ("dbg_kT1", BF16), ("dbg_xnT0", BF16), ("dbg_xnT1", BF16),
                        ("dbg_outT0", BF16), ("dbg_outT1", BF16)]:
            dbg[nm] = nc.dram_tensor(nm, [128, n], dt_, kind="ExternalOutput").ap()
        dbg["dbg_sums"] = nc.dram_tensor("dbg_sums", [2 * IB * 128, IBS], F32,
                                         kind="ExternalOutput").ap()
        dbg["dbg_R"] = nc.dram_tensor("dbg_R", [2 * IB * 128, IBS], F32,
                                      kind="ExternalOutput").ap()

    from contextlib import ExitStack

    with tile.TileContext(nc) as tc, ExitStack() as ctx:
        cp = ctx.enter_context(tc.tile_pool(name="const", bufs=1))

        def ctile(shape, dt, src, tag):
            t = cp.tile(shape, dt, tag=tag, name=tag)
            nc.sync.dma_start(t[:], src)
            return t

        posT = ctile([POS, n], BF16, posT_d[:, :], "posT")
        i128 = ctile([128, 128], F32, i128_d[:, :], "i128")
        onerb = ctile([1, 128], BF16, onerb_d[:, :], "onerb")
        wq = [[ctile([128, 128], BF16, wq_d[128 * k:128 * (k + 1), 128 * g:128 * (g + 1)],
                     f"wq{k}{g}") for g in range(2)] for k in range(3)]
        wk = [[ctile([128, 128], BF16, wk_d[128 * k:128 * (k + 1), 128 * g:128 * (g + 1)],
                     f"wk{k}{g}") for g in range(2)] for k in range(3)]
        wv = [ctile([128, DIM], BF16, wv_d[128 * k:128 * (k + 1), :], f"wv{k}") for k in range(2)]
        wp = [ctile([128, DIM], BF16, wp_d[p][:, :], f"wp{p}") for p in range(4)]
        bqT = ctile([1, DIM], BF16, bqT_d[:, :], "bqT")
        bkT = ctile([1, DIM], BF16, bkT_d[:, :], "bkT")
        oner512 = ctile([1, 512], BF16, oner512_d[:, :], "oner512")
        bv = ctile([1, DIM], BF16, bv_d[:, :], "bv")
        bp = ctile([1, DIM], BF16, bp_d[:, :], "bp")
        zeroc = ctile([128, 1], F32, zeroc_d[:, :], "zeroc")
        # all-zero bf16 [128,128]: lhsT for PE zero-fill of psum banks
        zcol = cp.tile([128, 128], BF16, tag="zcol", name="zcol")
        nc.vector.memset(zcol[:], 0.0)

        # persistent activations
        # x tiles are double-buffered by repeat parity so the next
        # iteration's x DMA + LN stats can run ahead while this iteration's
        # attention still holds the other buffer (the residual add in phase 5
        # is the last reader)
        nxb = 2 if repeat > 1 else 1
        xsb2 = [[cp.tile([128, DIM], F32, tag=f"x{r}_{t}", name=f"x{r}_{t}")
                 for t in range(TT)] for r in range(nxb)]
        xnT = [cp.tile([128, n], BF16, tag=f"xnT{g}", name=f"xnT{g}") for g in range(2)]
        qT = [cp.tile([128, n], BF16, tag=f"qT{g}", name=f"qT{g}") for g in range(2)]
        kT = [cp.tile([128, n], BF16, tag=f"kT{g}", name=f"kT{g}") for g in range(2)]
        # augmented V: per head h, cols 64h..64h+32 = v_h, cols 64h+32..64h+64 = 1.0
        vsb = [cp.tile([128, 8 * 64], BF16, tag=f"v{t}", name=f"v{t}") for t in range(TT)]
        for t in range(TT):
            nc.vector.memset(vsb[t][:], 1.0)
        # outT pair tiles: pair p rows = {O_h(2p) 0-31, junk, O_h(2p+1) 64-95, junk}
        outT = [cp.tile([128, n], BF16, tag=f"outT{p}", name=f"outT{p}") for p in range(4)]
        stats = cp.tile([128, 2 * TT], F32, tag="stats", name="stats")
        rstd = cp.tile([128, TT], F32, tag="rstd", name="rstd")
        # rsqrt Newton scratch (all [128, TT], tiny)
        vte = cp.tile([128, TT], F32, tag="vte", name="vte")
        rs_i = cp.tile([128, TT], I32, tag="rs_i", name="rs_i")
        rs_y = cp.tile([128, TT], I32, tag="rs_y", name="rs_y")
        rs_t = cp.tile([128, TT], F32, tag="rs_t", name="rs_t")
        rs_y1 = cp.tile([128, TT], F32, tag="rs_y1", name="rs_y1")
        magic_t = cp.tile([128, TT], I32, tag="magic", name="magic")
        nc.vector.memset(magic_t[:], RSQRT_MAGIC)

        bn6p = ctx.enter_context(tc.tile_pool(name="bn6", bufs=3))
        xcp = ctx.enter_context(tc.tile_pool(name="xc", bufs=3))
        ptp = ctx.enter_context(tc.tile_pool(name="pt", bufs=JT))
        rp = ctx.enter_context(tc.tile_pool(name="rsb", bufs=2))
        fp = ctx.enter_context(tc.tile_pool(name="fout", bufs=3))

        for _rep in range(repeat):
            xsb = xsb2[_rep % nxb]
            # ---------------- phase 1: LN stats ----------------
            for t in range(TT):
                nc.sync.dma_start(xsb[t][:], x_d[128 * t:128 * (t + 1), :])
                b6 = bn6p.tile([128, 6], F32, tag="b6", name="b6")
                nc.vector.bn_stats(b6[:], xsb[t][:])
                nc.vector.bn_aggr(stats[:, 2 * t:2 * t + 2], b6[:])

            # rstd = 1/sqrt(var+eps) via the fp32 bit-trick seed + two Newton
            # steps, entirely on DVE over tiny [128,16] tiles. This keeps ACT
            # down to Copy+Exp -- one table set for the whole program, zero
            # per-repeat table reloads (the old Sqrt cost two ~2.7us loads
            # per repeat).
            var_v = stats[:].rearrange("p (t c) -> p t c", c=2)[:, :, 1:2]
            nc.vector.tensor_scalar(vte[:].rearrange("p (t c) -> p t c", c=1),
                                    var_v, EPS, None, op0=Alu.add)
            nc.vector.tensor_scalar(rs_i[:], vte[:].bitcast(I32), 1, None,
                                    op0=Alu.arith_shift_right)
            nc.vector.tensor_sub(rs_y[:], magic_t[:], rs_i[:])
            y0 = rs_y[:].bitcast(F32)
            nc.vector.tensor_mul(rs_t[:], y0, y0)
            nc.vector.tensor_mul(rs_t[:], rs_t[:], vte[:])
            nc.vector.tensor_scalar(rs_t[:], rs_t[:], -0.5, 1.5,
                                    op0=Alu.mult, op1=Alu.add)
            nc.vector.tensor_mul(rs_y1[:], y0, rs_t[:])
            nc.vector.tensor_mul(rs_t[:], rs_y1[:], rs_y1[:])
            nc.vector.tensor_mul(rs_t[:], rs_t[:], vte[:])
            nc.vector.tensor_scalar(rs_t[:], rs_t[:], -0.5, 1.5,
                                    op0=Alu.mult, op1=Alu.add)
            nc.vector.tensor_mul(rstd[:], rs_y1[:], rs_t[:])

            with tc.tile_pool(name="proj_psum", bufs=2, space="PSUM") as pp:
                # ---------------- phase 2: center+scale, transpose ----------------
                # gamma/beta are folded into Wq/Wk/Wv host-side, so xnT is a
                # plain psum->sbuf copy; it goes on ACT (idle here) while DVE
                # runs stats/centering
                for t in range(TT):
                    xc = xcp.tile([128, DIM], F32, tag="xc", name="xc")
                    nc.vector.tensor_scalar(xc[:], xsb[t][:], stats[:, 2 * t:2 * t + 1],
                                            rstd[:, t:t + 1], op0=Alu.subtract, op1=Alu.mult)
                    for g in range(2):
                        tp = pp.tile([128, 128], F32, tag="tp", name="tp")
                        nc.tensor.transpose(tp[:], xc[:, 128 * g:128 * (g + 1)], i128[:])
                        (nc.vector.tensor_copy if "dve_xnt" in _x
                         else nc.scalar.copy)(xnT[g][:, 128 * t:128 * (t + 1)], tp[:])

                # ---------------- phase 3: qT / kT / v projections ----------------
                # biases are added on PE via K=1 ones matmuls, so the
                # psum->sbuf moves are pure copies (ACT for q, DVE for k,
                # balancing the two engines through this phase)
                for g in range(2):
                    for c in range(n512):
                        cs = slice(512 * c, 512 * (c + 1))
                        qp = pp.tile([128, 512], F32, tag="qk", name="qk")
                        for ki in range(3):
                            rhs = xnT[ki][:, cs] if ki < 2 else posT[:, cs]
                            nc.tensor.matmul(qp[:], lhsT=wq[ki][g][:], rhs=rhs,
                                             start=(ki == 0), stop=False)
                        nc.tensor.matmul(qp[:], lhsT=bqT[:, 128 * g:128 * (g + 1)],
                                         rhs=oner512[:], start=False, stop=True)
                        (nc.vector.tensor_copy if "dve_qcopy" in _x
                         else nc.scalar.copy)(qT[g][:, cs], qp[:])
                        kp = pp.tile([128, 512], F32, tag="qk", name="qk")
                        for ki in range(3):
                            rhs = xnT[ki][:, cs] if ki < 2 else posT[:, cs]
                            nc.tensor.matmul(kp[:], lhsT=wk[ki][g][:], rhs=rhs,
                                             start=(ki == 0), stop=False)
                        nc.tensor.matmul(kp[:], lhsT=bkT[:, 128 * g:128 * (g + 1)],
                                         rhs=oner512[:], start=False, stop=True)
                        (nc.vector.tensor_copy if "dve_kcopy" in _x
                         else nc.scalar.copy)(kT[g][:, cs], kp[:])

                for t in range(TT):
                    ts_ = slice(128 * t, 128 * (t + 1))
                    vp = pp.tile([128, DIM], F32, tag="vp", name="vp")
                    nc.tensor.matmul(vp[:], lhsT=xnT[0][:, ts_], rhs=wv[0][:],
                                     start=True, stop=False)
                    nc.tensor.matmul(vp[:], lhsT=xnT[1][:, ts_], rhs=wv[1][:],
                                     start=False, stop=False)
                    nc.tensor.matmul(vp[:], lhsT=onerb[:], rhs=bv[:], start=False, stop=True)
                    vdst = vsb[t][:].rearrange("p (h c) -> p h c", c=64)[:, :, 0:32]
                    (nc.vector.tensor_copy if "dve_vcopy" in _x
                     else nc.scalar.copy)(vdst, vp[:].rearrange("p (h c) -> p h c", c=32))

            # ---------------- phase 4: attention ----------------
            with tc.tile_pool(name="st_psum", bufs=1, space="PSUM") as stp, \
                 tc.tile_pool(name="os_psum", bufs=2, space="PSUM") as op:
                def emit_exp(src, dst, on_dve):
                    if "no_exp" in _x:
                        return
                    if on_dve and "small_exp" not in _x:
                        # Schraudolph exp on DVE: bf16 bit pattern built by
                        # int16 round of EXPA*S+EXPB
                        nc.vector.tensor_scalar(dst.bitcast(I16), src,
                                                EXPA, EXPB,
                                                op0=Alu.mult, op1=Alu.add)
                    else:
                        ncols = 256 if "small_exp" in _x else 2 * IBS
                        nc.scalar.activation(dst[:, :ncols], src[:, :ncols],
                                             Act.Exp, bias=zeroc[:], scale=SCALE)

                for g in range(2):
                    for ib in range(IB):
                        ibs = slice(IBS * ib, IBS * (ib + 1))
                        osA = op.tile([128, IBS], F32, tag="osA", name="osA")
                        osB = op.tile([128, IBS], F32, tag="osB", name="osB")
                        # zero-fill the accumulator banks on PE (start=True
                        # with an all-zero lhsT), keeping DVE free for exps
                        if "dve_memset" in _x:
                            nc.vector.memset(osA[:], 0.0)
                            nc.vector.memset(osB[:], 0.0)
                        else:
                            nc.tensor.matmul(osA[:], lhsT=zcol[:], rhs=kT[g][:, ibs],
                                             start=True, stop=False)
                            nc.tensor.matmul(osB[:], lhsT=zcol[:], rhs=kT[g][:, ibs],
                                             start=True, stop=False)
                        for jt in range(JT):
                            # two 2-head S^T tiles: exp(stA) overlaps matmuls into stB
                            stA = stp.tile([128, 2 * IBS], F32, tag="stA", name="stA")
                            stB = stp.tile([128, 2 * IBS], F32, tag="stB", name="stB")
                            pt = ptp.tile([128, 4 * IBS], BF16, tag="pt", name="pt")
                            for h in range(4):
                                sth = (stA, stB)[h // 2]
                                nc.tensor.matmul(
                                    sth[:, IBS * (h % 2):IBS * (h % 2 + 1)],
                                    lhsT=kT[g][32 * h:32 * (h + 1), 128 * jt:128 * (jt + 1)],
                                    rhs=qT[g][32 * h:32 * (h + 1), ibs],
                                    start=True, stop=True, tile_position=(32 * h, 0))
                                if h == 1:
                                    emit_exp(stA[:, :2 * IBS], pt[:, :2 * IBS],
                                             jt in dve_a)
                            emit_exp(stB[:, :2 * IBS], pt[:, 2 * IBS:4 * IBS],
                                     jt in dve_b)
                            if "no_exp" in _x:
                                nc.vector.memset(pt[:, :1], 1.0)  # keep pt initialized
                            # fused O+sums: lhsT = [v_h | ones32] (M=64); adjacent
                            # matmuls use distinct banks AND distinct col groups so
                            # both the array and the psum drains run concurrently:
                            #   h0->(A,0) h1->(B,64) h2->(A,64) h3->(B,0)
                            for h in range(4) if "no_osums" not in _x else []:
                                bank = (osA, osB)[h % 2]
                                pos = (0, 64, 64, 0)[h]
                                nc.tensor.matmul(
                                    bank[pos:pos + 64, :],
                                    lhsT=vsb[jt][:, 64 * (4 * g + h):64 * (4 * g + h) + 64],
                                    rhs=pt[:, IBS * h:IBS * (h + 1)],
                                    start=False, stop=(jt == JT - 1),
                                    tile_position=(0, pos), skip_group_check=True)
                        # normalization: bank rows {0-31 O_a, 32-63 r_a, 64-95 O_b,
                        # 96-127 r_b}; recip the whole bank, then partition-shift
                        # the recip rows down by 32 via SBUF->SBUF DMA so they
                        # align with the O rows (also duplicated onto their own
                        # rows so junk rows stay finite: r * 1/r ~ 1).
                        for bi, bank in enumerate((osA, osB)):
                            r_sb = rp.tile([128, IBS], F32, tag="r", name="r")
                            if use_recip_approx:
                                nc.vector.reciprocal_approx_fast(r_sb[:], bank[:])
                            else:
                                nc.vector.reciprocal(r_sb[:], bank[:])
                            r_al = rp.tile([128, IBS], F32, tag="ral", name="ral")
                            for half in (0, 64):
                                src = r_sb[half + 32:half + 64, :]
                                nc.sync.dma_start(r_al[half:half + 32, :], src)
                                nc.sync.dma_start(r_al[half + 32:half + 64, :], src)
                            p = 2 * g + bi
                            nc.vector.tensor_mul(outT[p][:, ibs], bank[:], r_al[:])
                            if debug_dump:
                                it = (g * IB + ib)
                                if bi == 0:
                                    nc.sync.dma_start(
                                        dbg["dbg_R"][128 * it:128 * (it + 1), :], r_al[:])

                        # ---- phase 5 for this i-block: once g=1 is
                        # normalized, all four outT pairs cover these tokens,
                        # so the output projection + residual + store overlap
                        # the next i-block's attention ----
                        if g == 1:
                            for t in range(4 * ib, 4 * ib + 4):
                                ts_ = slice(128 * t, 128 * (t + 1))
                                f_ps = op.tile([128, DIM], F32, tag="osA", name="fo")
                                for p in range(4):
                                    nc.tensor.matmul(f_ps[:], lhsT=outT[p][:, ts_],
                                                     rhs=wp[p][:],
                                                     start=(p == 0), stop=False)
                                nc.tensor.matmul(f_ps[:], lhsT=onerb[:], rhs=bp[:],
                                                 start=False, stop=False)
                                # residual add on PE: exact fp32 identity
                                # matmul into the same accumulation group,
                                # then a plain ACT copy -- keeps DVE free
                                # for exp work
                                nc.tensor.matmul(f_ps[:], lhsT=i128[:],
                                                 rhs=xsb[t][:],
                                                 start=False, stop=True)
                                f_sb = fp.tile([128, DIM], F32, tag="f", name="f")
                                if "dve_resid" in _x:
                                    nc.vector.tensor_copy(f_sb[:], f_ps[:])
                                else:
                                    nc.scalar.copy(f_sb[:], f_ps[:])
                                nc.sync.dma_start(out_d[ts_, :], f_sb[:])

                if debug_dump:
                    nc.sync.dma_start(dbg["dbg_qT0"][:, :], qT[0][:])
                    nc.sync.dma_start(dbg["dbg_qT1"][:, :], qT[1][:])
                    nc.sync.dma_start(dbg["dbg_kT0"][:, :], kT[0][:])
                    nc.sync.dma_start(dbg["dbg_kT1"][:, :], kT[1][:])
                    nc.sync.dma_start(dbg["dbg_xnT0"][:, :], xnT[0][:])
                    nc.sync.dma_start(dbg["dbg_xnT1"][:, :], xnT[1][:])
                    nc.sync.dma_start(dbg["dbg_outT0"][:, :], outT[0][:])
                    nc.sync.dma_start(dbg["dbg_outT1"][:, :], outT[2][:])

    nc.compile()
    return nc


def make_in_maps(inputs, n=N, nb=B):
    x = np.ascontiguousarray(np.asarray(inputs["x"], np.float32))
    pos = np.asarray(inputs["pos_embed"], np.float32)
    f32 = lambda a: np.ascontiguousarray(np.asarray(a, np.float32))
    bf16 = lambda a: np.ascontiguousarray(np.asarray(a, np.float32).astype(BF16_NP))

    # fold layernorm gamma/beta into the projection weights and biases:
    # xn = c*g + b with c = (x-mu)*rstd, so W'[:DIM] = g[:,None]*W[:DIM] and
    # bias' = bias + b @ W[:DIM]
    g_ln = f32(inputs["ln_g"])
    b_ln = f32(inputs["ln_b"])
    Wq = f32(inputs["Wq"]).copy()
    Wk = f32(inputs["Wk"]).copy()
    Wv = f32(inputs["Wv"]).copy()
    bqe = f32(inputs["bq"]) + b_ln @ Wq[:DIM]
    bke = f32(inputs["bk"]) + b_ln @ Wk[:DIM]
    bve = f32(inputs["bv"]) + b_ln @ Wv
    Wq[:DIM] *= g_ln[:, None]
    Wk[:DIM] *= g_ln[:, None]
    Wv *= g_ln[:, None]
    shared = {
        "wq": bf16(Wq), "wk": bf16(Wk), "wv": bf16(Wv),
        "bqT": bf16(bqe).reshape(1, DIM), "bkT": bf16(bke).reshape(1, DIM),
        "bv": bf16(bve).reshape(1, DIM), "bp": bf16(inputs["bp"]).reshape(1, DIM),
        "ones_row_bf": np.ones((1, 128), BF16_NP),
        "ones_row_512": np.ones((1, 512), BF16_NP),
        "i128": np.eye(128, dtype=np.float32),
        "zeroc": np.zeros((128, 1), np.float32),
    }
    wp_full = f32(inputs["Wp"])
    # pair tile p=2g+bi: bankA(bi=0) holds heads (4g+0, 4g+2); bankB holds (4g+3, 4g+1)
    pair_heads = [(0, 2), (3, 1), (4, 6), (7, 5)]
    for p, (ha, hb) in enumerate(pair_heads):
        pad = np.zeros((128, DIM), np.float32)
        pad[0:32] = wp_full[32 * ha:32 * ha + 32]
        pad[64:96] = wp_full[32 * hb:32 * hb + 32]
        shared[f"wp{p}"] = pad.astype(BF16_NP)
    in_maps = []
    for b in range(nb):
        m = dict(shared)
        m["x"] = np.ascontiguousarray(x[b, :n])
        m["posT"] = np.ascontiguousarray(pos[b, :n].T.astype(BF16_NP))
        in_maps.append(m)
    return in_maps


_NC_CACHE = {}


def kernel(**inputs):
    if "nc" not in _NC_CACHE:
        _NC_CACHE["nc"] = build_nc()
    nc = _NC_CACHE["nc"]
    in_maps = make_in_maps(inputs)
    trace = bool(int(os.environ.get("KERNEL_TRACE", "0")))
    if not trace:
        # the axon NTFF trace hook is absent in this deployment; a stray
        # BASS_TRACE in the caller's env would crash run_bass_kernel_spmd
        os.environ["BASS_NEVER_TRACE"] = "1"
    res = bass_utils.run_bass_kernel_spmd(nc, in_maps, core_ids=list(range(B)), trace=trace)
    kernel.last_results = res
    kernel.last_exec_time_ns = res.exec_time_ns
    return np.stack([r["out"] for r in res.results]).astype(np.float32)

